# revision 61
# baseline (speedup 1.0000x reference)
"""BiaffineLabelAttention kernel for 8 TRN2 NeuronCores (Bass/Tile).

Reference computation (per full input):
    t1[b,l,i,o] = sum_d head[b,i,d] * U[l,d] * dep[b,o,d]
    t2_h[b,l,i] = sum_d W_h[l,d] * head[b,i,d]
    t2_d[b,l,o] = sum_d W_d[l,d] * dep[b,o,d]
    out = t1 + t2_h[...,None] + t2_d[...,None,:] + bias[l]

Sharding: data-parallel over batch (16 batches -> 2 per core x 8 cores).

Host pre-formats inputs (layout only): head/dep are transposed to
[b, d, i] so the kernel needs no PE transposes; the label weights are
pre-arranged into the [128, KT*L] SBUF layout.

Per-core algorithm (fp32r matmuls, fp32 PSUM accumulation):
    scaled[d,(l,o)] = U[l,d]*depT[d,o] + W_h[l,d]      (DVE/ACT/Pool fma)
    psum[i,(l,o)]  += headT[d,i].T @ scaled[d,(l,o)]   (fp32r matmuls, N=512)
    out_sb(bf16) = psum + t2row[(l,o)]                  (DVE tensor_tensor add)
The W_h term telescopes: sum_d headT[d,i]*W_h[l,d] = t2_h[b,l,i].
t2row = t2_d + bias comes from a small PE matmul bounced through DRAM and
replicated across partitions by 0-stride-broadcast DMA reads (the DMA
engines have slack; the Pool engine does not). Output is written bf16
and upcast on host. Output DMAs cover two label pairs each and rotate
across the SP/ACT queues.
"""

import numpy as np
import ml_dtypes
from contextlib import ExitStack

import concourse.bass as bass
from concourse import bacc, mybir, tile
from concourse.bass_utils import run_bass_kernel_spmd

F32 = mybir.dt.float32
F32R = mybir.dt.float32r
BF16 = mybir.dt.bfloat16

B, S, D, L = 16, 256, 768, 32
NCORES = 8
BC = B // NCORES          # batches per core
KT = D // 128             # contraction k-tiles
PAIRS = L // 2            # label pairs sharing one PSUM bank (N=512)
GROUPS = PAIRS // 2       # output-DMA groups (2 pairs each)
ROWLEN = L * S            # per-batch t2 row length (l,o) flattened

_NC_CACHE = {}


def _build_nc():
    nc = bacc.Bacc(
        "TRN2",
        target_bir_lowering=False,
        debug=False,
        enable_asserts=False,
        num_devices=NCORES,
    )
    # host-transposed inputs laid out as the exact SBUF image:
    # head_t[p, (b*KT+k)*S + i] = head[b, i, k*128+p]  (contiguous per
    # partition -> one long DMA run per partition per load call)
    headt_d = nc.dram_tensor("headt", [128, BC * KT * S], F32, kind="ExternalInput")
    dept_d = nc.dram_tensor("dept", [128, BC * KT * S], F32, kind="ExternalInput")
    # weights pre-arranged: col k*L+l on partition p holds X[l, k*128+p]
    ut_d = nc.dram_tensor("ut", [128, KT * L], F32, kind="ExternalInput")
    wht_d = nc.dram_tensor("wht", [128, KT * L], F32, kind="ExternalInput")
    wdt_d = nc.dram_tensor("wdt", [128, KT * L], F32, kind="ExternalInput")
    b_d = nc.dram_tensor("b", [L, 1], F32, kind="ExternalInput")
    out_d = nc.dram_tensor("out", [BC, L, S, S], BF16, kind="ExternalOutput")
    t2_scratch = nc.dram_tensor("t2_scratch", [BC, L, S], BF16)

    with tile.TileContext(nc) as tc, ExitStack() as ctx:
        const = ctx.enter_context(tc.tile_pool(name="const", bufs=1))
        big = ctx.enter_context(tc.tile_pool(name="big", bufs=1))
        nat = ctx.enter_context(tc.tile_pool(name="nat", bufs=2))
        scaled_pool = ctx.enter_context(tc.tile_pool(name="scaled", bufs=30))
        outp = ctx.enter_context(tc.tile_pool(name="outp", bufs=4))
        mm_psum = ctx.enter_context(
            tc.tile_pool(name="mm_psum", bufs=8, space=bass.MemorySpace.PSUM)
        )

        headT = big.tile([128, BC * KT * S], F32R, tag="headT")  # [d, (b,k,i)]
        depT = big.tile([128, BC * KT * S], F32R, tag="depT")    # [d, (b,k,o)]
        ut = big.tile([128, KT * L], F32, tag="ut")    # col k*L+l = U[l, k-blk]
        wht = big.tile([128, KT * L], F32, tag="wht")
        wdt = big.tile([128, KT * L], F32R, tag="wdt")
        bias = const.tile([L, 1], F32, tag="bias")
        # t2bc[p, b*ROWLEN + l*S + o] = t2_d[b,l,o] + bias[l], all partitions
        t2bc = big.tile([128, BC * ROWLEN], BF16, tag="t2bc")

        def col(b, k):
            return (b * KT + k) * S

        def load_t(eng, src_d, dst, b, k0, nk):
            c0, c1 = col(b, k0), col(b, k0) + nk * S
            dst_ap = dst[:, c0:c1]
            src = src_d[:, c0:c1]
            if src.dtype != dst_ap.dtype:
                src = src.bitcast(dst_ap.dtype)
            eng.dma_start(dst_ap, src)

        def t2_chain(b):
            # t2row[b] = t2_d[b] + bias -> DRAM scratch; per-chunk
            # 0-stride-broadcast DMA reads replicate it across partitions
            # just-in-time inside main_pairs (t2_bcast below)
            psf = mm_psum.tile([128, 2 * S], F32, tag="mm")
            ps = psf[:L, :S]
            for k in range(KT):
                nc.tensor.matmul(
                    ps,
                    wdt[:, k * L:(k + 1) * L],
                    depT[:, col(b, k):col(b, k) + S],
                    start=(k == 0),
                    stop=(k == KT - 1),
                )
            t2sb = nat.tile([L, S], BF16, tag="t2sb")
            # on ACT, not DVE: keeps the in-order DVE queue free of a
            # long-latency dependency on the t2 psum
            nc.scalar.activation(
                t2sb[:], ps, mybir.ActivationFunctionType.Identity,
                bias=bias[:],
            )
            eng = nc.scalar if b == 0 else nc.sync
            eng.dma_start(t2_scratch[b], t2sb[:])

        def t2_bcast(b, jj):
            # replicate t2row chunk (2 pairs = 4 labels) to all partitions
            src = t2_scratch[b, 4 * jj:4 * jj + 4, :].rearrange(
                "l o -> (l o)").partition_broadcast(128)
            c = b * ROWLEN + jj * 4 * S
            nc.sync.dma_start(t2bc[:, c:c + 4 * S], src)

        op_idx = [0]

        def scale_op(dst_ap, src_ap, ucol, wcol):
            # scaled = U*depT + W_h; rotate ACT/DVE/Pool 7:4:5 per 16 ops
            # (ratios set from measured HW per-op costs: 498/432/695 ns)
            i = op_idx[0] % 16
            op_idx[0] += 1
            if i in (3, 7, 11, 14, 15):
                nc.gpsimd.tensor_scalar(
                    dst_ap, src_ap, ucol, wcol,
                    mybir.AluOpType.mult, mybir.AluOpType.add,
                )
            elif i in (1, 5, 9, 13):
                nc.vector.tensor_scalar(
                    dst_ap, src_ap, ucol, wcol,
                    mybir.AluOpType.mult, mybir.AluOpType.add,
                )
            else:
                nc.scalar.activation(
                    dst_ap, src_ap,
                    mybir.ActivationFunctionType.Identity,
                    bias=wcol, scale=ucol,
                )

        dma_rr = [0]

        def out_dma(dst_ap, src_ap):
            # rotate output DMA issuance across the SP/ACT sequencers
            eng = (nc.sync, nc.scalar)[dma_rr[0] % 2]
            dma_rr[0] += 1
            eng.dma_start(dst_ap, src_ap)

        def main_pairs(b, split_last=False, at_group=None, at_pair=None,
                       skip_bcast0=False):
            for jj in range(GROUPS):
                if jj > 0 or not skip_bcast0:
                    t2_bcast(b, jj)
                if at_group is not None and jj in at_group:
                    at_group[jj]()
                # osb holds two pairs: free dims (j, l, ib, o)
                osb = outp.tile([128, 8 * S], BF16, tag="osb")
                osb5 = osb[:].rearrange(
                    "i (j l ib o) -> i j l ib o", j=2, l=2, ib=2)
                for jo in range(2):
                    j = 2 * jj + jo
                    stiles = []
                    for k in range(KT):
                        st = scaled_pool.tile([128, 2 * S], F32R, tag="scaled")
                        for h in range(2):
                            lbl = 2 * j + h
                            scale_op(
                                st[:, h * S:(h + 1) * S],
                                depT[:, col(b, k):col(b, k) + S],
                                ut[:, k * L + lbl:k * L + lbl + 1],
                                wht[:, k * L + lbl:k * L + lbl + 1],
                            )
                        stiles.append(st)
                    if at_pair is not None and j in at_pair:
                        at_pair[j]()
                    for ib in range(2):
                        ps = mm_psum.tile([128, 2 * S], F32, tag="mm")
                        for k in range(KT):
                            hc = col(b, k) + ib * 128
                            nc.tensor.matmul(
                                ps[:],
                                headT[:, hc:hc + 128],
                                stiles[k][:],
                                start=(k == 0),
                                stop=(k == KT - 1),
                            )
                        nc.vector.tensor_tensor(
                            osb5[:, jo, :, ib, :],
                            ps[:].rearrange("i (l o) -> i l o", l=2),
                            t2bc[:, b * ROWLEN + j * 2 * S:
                                 b * ROWLEN + (j + 1) * 2 * S].rearrange(
                                     "p (l o) -> p l o", l=2),
                            mybir.AluOpType.add,
                        )
                if split_last and jj >= GROUPS - 2:
                    # split the final group into 8 small DMAs for a fast tail
                    for c in range(8):
                        jl, ib = c // 2, c % 2
                        out_dma(
                            out_d[b, 4 * jj + jl,
                                  ib * 128:(ib + 1) * 128, :],
                            osb[:, c * S:(c + 1) * S],
                        )
                else:
                    # one DMA per (b, 2 pairs): HBM dims (j,l,ib) uniform
                    out_dma(
                        out_d[b, 4 * jj:4 * jj + 4, :, :].rearrange(
                            "(j l) (ib i) o -> i (j l ib) o", l=2, i=128),
                        osb[:].rearrange("i (jlib o) -> i jlib o", jlib=8),
                    )

        # PE warm-up: dummy matmuls on a memset tile while the input DMAs
        # are in flight, so the PE p-state is at full clock (and the first
        # real matmul isn't paying the ramp) when real work arrives
        wu = nat.tile([128, 2 * S], F32, tag="wu")
        nc.gpsimd.memset(wu[:], 0)
        for _ in range(22):
            wps = mm_psum.tile([128, 2 * S], F32, tag="mm")
            nc.tensor.matmul(wps[:], wu[:, :128].bitcast(F32R),
                             wu[:].bitcast(F32R), start=True, stop=True)

        load_t(nc.sync, dept_d, depT, 0, 0, 1)
        nc.scalar.dma_start(ut[:], ut_d[:])
        nc.scalar.dma_start(wht[:], wht_d[:])
        load_t(nc.scalar, headt_d, headT, 0, 0, 1)
        load_t(nc.sync, dept_d, depT, 0, 1, 2)
        load_t(nc.scalar, headt_d, headT, 0, 1, 2)
        nc.sync.dma_start(wdt[:], wdt_d[:].bitcast(F32R))
        nc.sync.dma_start(bias[:], b_d[:])
        load_t(nc.sync, dept_d, depT, 0, 3, 3)
        load_t(nc.scalar, headt_d, headT, 0, 3, 3)
        load_t(nc.sync, dept_d, depT, 1, 0, 6)
        load_t(nc.sync, headt_d, headT, 1, 0, 6)
        # t2 chains are deferred into the main stream: their matmuls need
        # the FULL depT of their batch and would otherwise sit at the head
        # of the in-order PE queue, stalling the whole main stream
        def _chain0():
            t2_chain(0)
            t2_bcast(0, 0)

        main_pairs(0, at_pair={0: _chain0}, skip_bcast0=True,
                   at_group={2: lambda: t2_chain(1)})
        main_pairs(1, split_last=True)

    nc.compile()
    return nc


def get_nc():
    if "nc" not in _NC_CACHE:
        _NC_CACHE["nc"] = _build_nc()
    return _NC_CACHE["nc"]


def make_in_maps(head, dep, u, w, bvec):
    head = np.asarray(head, dtype=np.float32)
    dep = np.asarray(dep, dtype=np.float32)
    u = np.asarray(u, dtype=np.float32)
    w = np.asarray(w, dtype=np.float32)
    bcol = np.ascontiguousarray(
        np.asarray(bvec, dtype=np.float32).reshape(L, 1)
    )
    # [B,S,D] -> [128, B*KT*S] SBUF image (host-side layout prep):
    # tform(x)[p, (b*KT+k)*S + i] = x[b, i, k*128+p]
    def tform(x, dt=np.float32):
        xt = x.transpose(0, 2, 1).reshape(B, KT, 128, S)
        xt = xt.transpose(2, 0, 1, 3).reshape(128, B * KT * S)
        return np.ascontiguousarray(xt.astype(dt))

    headt = tform(head)
    dept = tform(dep)
    # [L, D] -> [128, KT*L]: col k*L+l on partition p = X[l, k*128+p]
    def wform(x):
        xt = x.reshape(L, KT, 128).transpose(2, 1, 0).reshape(128, KT * L)
        return np.ascontiguousarray(xt)

    ut = wform(u)
    wht = wform(w[:, :D])
    wdt = wform(w[:, D:])
    cw = BC * KT * S
    return [
        {
            "headt": np.ascontiguousarray(headt[:, c * cw:(c + 1) * cw]),
            "dept": np.ascontiguousarray(dept[:, c * cw:(c + 1) * cw]),
            "ut": ut,
            "wht": wht,
            "wdt": wdt,
            "b": bcol,
        }
        for c in range(NCORES)
    ]


def run(head, dep, label_U_diag, label_W, label_b, trace=False, **trace_kw):
    nc = get_nc()
    in_maps = make_in_maps(head, dep, label_U_diag, label_W, label_b)
    res = run_bass_kernel_spmd(
        nc, in_maps, core_ids=list(range(NCORES)), trace=trace, **trace_kw
    )
    out = np.concatenate(
        [np.asarray(res.results[c]["out"]).astype(np.float32)
         for c in range(NCORES)],
        axis=0,
    )
    return out, res


def kernel(**inputs):
    out, _ = run(
        inputs["head"],
        inputs["dep"],
        inputs["label_U_diag"],
        inputs["label_W"],
        inputs["label_b"],
    )
    return out
